# revision 1
# baseline (speedup 1.0000x reference)
"""FP64->FP32 bit-circuit converter kernel for Trainium2 (8 NeuronCores).

Input:  fp64_pulse (1048576, 64) float32 of {0,1} bits (fp64, MSB first).
Output: (1048576, 32) float32 of {0,1} bits (fp32 conversion result).

Strategy (pure data parallel over batch, 131072 rows/core):
  - batch-2D layout: 128 partitions x 1024 inner rows per core,
    supertiles of NF inner rows ([128, NF*64] input tiles),
  - bit packing into integers via one broadcast-weight multiply and
    innermost-axis reduces (exp_val, mant_int, sticky_sum),
  - mantissa round-to-nearest-even done by fp32 hardware: adding
    frac = 0.5*R + 0.25*S to the 24-bit int (2^23 + mant_int) rounds
    exactly like the reference ripple-adder circuit,
  - special values (nan/inf/overflow/underflow) folded in as value-level
    muxes, then output bits extracted with int32 (shift, and) ops.
"""
import numpy as np

from concourse import bacc, mybir
from concourse.tile import TileContext
from concourse.bass_utils import run_bass_kernel_spmd

AOT = mybir.AluOpType
F32 = mybir.dt.float32
BF16 = mybir.dt.bfloat16
I32 = mybir.dt.int32

B = 1_048_576
N_CORES = 8
B_CORE = B // N_CORES          # 131072
P = 128                        # partitions
NI = B_CORE // P               # 1024 inner rows per partition
NF = 128                       # inner rows per supertile
N_ST = NI // NF                # supertiles per core
D_IN = 64
D_OUT = 32

# weight row: col 0 sign (unused), cols 1..11 exp MSB-first (2^10..2^0),
# cols 12..34 mant bits 0..22 MSB-first (2^22..2^0), rest 0.
_w_row = np.zeros(D_IN, np.float32)
_w_row[1:12] = [2.0 ** (10 - k) for k in range(11)]
_w_row[12:35] = [2.0 ** (22 - k) for k in range(23)]
import ml_dtypes
WCONST = np.broadcast_to(_w_row, (P, D_IN)).astype(ml_dtypes.bfloat16).copy()

_CACHE = {}


def _build():
    nc = bacc.Bacc("TRN2")
    x = nc.dram_tensor("x", [B_CORE, D_IN], F32, kind="ExternalInput")
    w = nc.dram_tensor("w", [P, D_IN], BF16, kind="ExternalInput")
    y = nc.dram_tensor("y", [B_CORE, D_OUT], F32, kind="ExternalOutput")

    x_r = x.ap().rearrange("(p n) d -> p (n d)", p=P)   # [128, NI*64]
    y_r = y.ap().rearrange("(p n) d -> p (n d)", p=P)   # [128, NI*32]

    with TileContext(nc) as tc:
        with (
            tc.tile_pool(name="consts", bufs=1) as consts,
            tc.tile_pool(name="io", bufs=2) as io,
            tc.tile_pool(name="mid", bufs=2) as mid,
            tc.tile_pool(name="sc", bufs=3) as sc,
        ):
            wt = consts.tile([P, D_IN], BF16)
            nc.sync.dma_start(wt[:, :], w.ap())

            # small first/last supertiles shorten the DMA head/tail ramp
            schedule = [32, 96] + [NF] * (N_ST - 2) + [96, 32]
            assert sum(schedule) == NI
            off = 0
            for nf in schedule:
                xin = io.tile([P, nf * D_IN], F32, tag="xin")
                nc.sync.dma_start(
                    xin[:, :], x_r[:, off * D_IN:(off + nf) * D_IN])

                xv = xin[:, :].rearrange("p (n d) -> p n d", d=D_IN)

                # --- weighted pack: mult by broadcast weights + reduces ---
                wb = wt[:, 1:35].unsqueeze(1).broadcast_to([P, nf, 34])
                xw = mid.tile([P, nf * 34], F32, tag="xw")
                xwv = xw[:, :].rearrange("p (n d) -> p n d", d=34)
                nc.vector.tensor_tensor(xwv, xv[:, :, 1:35], wb, AOT.mult)

                exp_val_t = sc.tile([P, nf], F32, tag="exp_val")
                nc.vector.tensor_reduce(
                    exp_val_t[:, :].unsqueeze(2), xwv[:, :, 0:11],
                    mybir.AxisListType.X, AOT.add)
                mant_int_t = sc.tile([P, nf], F32, tag="mant_int")
                nc.vector.tensor_reduce(
                    mant_int_t[:, :].unsqueeze(2), xwv[:, :, 11:34],
                    mybir.AxisListType.X, AOT.add)
                sticky_t = sc.tile([P, nf], F32, tag="sticky")
                nc.vector.tensor_reduce(
                    sticky_t[:, :].unsqueeze(2), xv[:, :, 36:64],
                    mybir.AxisListType.X, AOT.add)
                exp_val = exp_val_t[:, :]
                mant_int = mant_int_t[:, :]
                sticky = sticky_t[:, :]
                Rbit = xv[:, :, 35]
                sign = xv[:, :, 0]

                # --- mantissa RNE via hw fp32 add ---
                fr2 = sc.tile([P, nf], BF16, tag="fr2")
                nc.any.tensor_scalar(fr2[:, :], sticky, 1.0, 0.25,
                                     AOT.is_ge, AOT.mult)
                frac = sc.tile([P, nf], BF16, tag="frac")
                nc.vector.scalar_tensor_tensor(frac[:, :], Rbit, 0.5, fr2[:, :],
                                               AOT.mult, AOT.add)
                Mr = sc.tile([P, nf], F32, tag="Mr")
                nc.vector.scalar_tensor_tensor(Mr[:, :], mant_int,
                                               float(2 ** 23), frac[:, :],
                                               AOT.add, AOT.add)
                c_m = sc.tile([P, nf], BF16, tag="c_m")
                nc.any.tensor_scalar(c_m[:, :], Mr[:, :], float(2 ** 24),
                                     None, AOT.is_ge)
                # Mval = Mr - 2^23*c_m  (in [2^23, 2^24); bit 23 never read)
                Mval = sc.tile([P, nf], F32, tag="Mval")
                nc.vector.scalar_tensor_tensor(Mval[:, :], c_m[:, :],
                                               float(-(2 ** 23)), Mr[:, :],
                                               AOT.mult, AOT.add)

                # --- exponent value T2 = exp_val + c_m + 1152 ---
                T2 = sc.tile([P, nf], F32, tag="T2")
                nc.vector.scalar_tensor_tensor(T2[:, :], c_m[:, :], 1152.0,
                                               exp_val, AOT.add, AOT.add)

                # --- specials ---
                over = sc.tile([P, nf], BF16, tag="over")
                nc.any.tensor_scalar(over[:, :], exp_val, 1151.0, None,
                                     AOT.is_ge)
                under = sc.tile([P, nf], BF16, tag="under")
                nc.any.tensor_scalar(under[:, :], exp_val, 897.0, None,
                                     AOT.is_lt)
                # m_any = (mant_int + 2*frac) >= 0.5  (frac = 0.5R + 0.25S)
                ms1 = sc.tile([P, nf], F32, tag="ms1")
                nc.vector.scalar_tensor_tensor(ms1[:, :], frac[:, :], 2.0,
                                               mant_int, AOT.mult, AOT.add)
                m_any = sc.tile([P, nf], F32, tag="m_any")
                nc.any.tensor_scalar(m_any[:, :], ms1[:, :], 0.5, None, AOT.is_ge)
                is_max = sc.tile([P, nf], BF16, tag="is_max")
                nc.any.tensor_scalar(is_max[:, :], exp_val, 2047.0, None,
                                     AOT.is_ge)
                # nan = is_max AND m_any, via sum >= 2 (fused into nv below)
                nan2 = sc.tile([P, nf], F32, tag="nan2")
                nc.vector.tensor_tensor(nan2[:, :], is_max[:, :], m_any[:, :],
                                        AOT.add)
                ou = sc.tile([P, nf], BF16, tag="ou")
                nc.vector.tensor_tensor(ou[:, :], over[:, :], under[:, :],
                                        AOT.add)
                Acoef = sc.tile([P, nf], BF16, tag="Acoef")
                nc.any.tensor_scalar(Acoef[:, :], ou[:, :], -1.0, 1.0,
                                     AOT.mult, AOT.add)

                # --- value-level muxes ---
                VV = sc.tile([P, 2 * nf], F32, tag="VV")
                Vexp = VV[:, 0:nf]
                Vm = VV[:, nf:2 * nf]
                vx = sc.tile([P, nf], F32, tag="vx")
                nc.vector.tensor_tensor(vx[:, :], T2[:, :], Acoef[:, :],
                                        AOT.mult)
                nc.vector.scalar_tensor_tensor(Vexp, over[:, :], 255.0,
                                               vx[:, :], AOT.mult, AOT.add)
                nv = sc.tile([P, nf], BF16, tag="nv")
                nc.any.tensor_scalar(nv[:, :], nan2[:, :], 2.0, float(2 ** 22),
                                     AOT.is_ge, AOT.mult)
                mx = sc.tile([P, nf], F32, tag="mx")
                nc.vector.tensor_tensor(mx[:, :], Mval[:, :], Acoef[:, :],
                                        AOT.mult)
                nc.vector.tensor_tensor(Vm, mx[:, :], nv[:, :], AOT.add)

                # --- bit extraction (int32); one wide convert for both ---
                VVi = sc.tile([P, 2 * nf], I32, tag="VVi")
                nc.any.tensor_copy(VVi[:, :], VV[:, :])
                Vexpi = VVi[:, 0:nf]
                Vmi = VVi[:, nf:2 * nf]

                yti = mid.tile([P, nf * D_OUT], I32, tag="yti")
                ytv = yti[:, :].rearrange("p (n d) -> p n d", d=D_OUT)
                # sign (col 0): f32 -> i32 convert copy
                nc.any.tensor_copy(ytv[:, :, 0], sign)
                # exp cols 1..8 = T2 bits 7..0
                for c in range(1, 9):
                    j = 8 - c
                    nc.any.tensor_scalar(ytv[:, :, c], Vexpi, j, 1,
                                         AOT.logical_shift_right,
                                         AOT.bitwise_and)
                # mant cols 9..31 = M bits 22..0
                for c in range(9, 32):
                    j = 31 - c
                    nc.any.tensor_scalar(ytv[:, :, c], Vmi, j, 1,
                                         AOT.logical_shift_right,
                                         AOT.bitwise_and)

                # int32 -> f32 convert in place via bitcast view, then DMA out
                ytf = yti[:, :].bitcast(F32)
                nc.any.tensor_copy(ytf, yti[:, :])
                nc.sync.dma_start(
                    y_r[:, off * D_OUT:(off + nf) * D_OUT], ytf)
                off += nf

    nc.compile()
    return nc


def _get_nc():
    if "nc" not in _CACHE:
        _CACHE["nc"] = _build()
    return _CACHE["nc"]


def kernel(fp64_pulse: np.ndarray) -> np.ndarray:
    x = np.ascontiguousarray(fp64_pulse, dtype=np.float32)
    assert x.shape == (B, D_IN)
    nc = _get_nc()
    in_maps = [
        {"x": x[c * B_CORE:(c + 1) * B_CORE], "w": WCONST}
        for c in range(N_CORES)
    ]
    res = run_bass_kernel_spmd(nc, in_maps, core_ids=list(range(N_CORES)))
    return np.concatenate([r["y"] for r in res.results], axis=0)



# revision 9
# speedup vs baseline: 3.8358x; 3.8358x over previous
"""FP64->FP32 bit-circuit converter kernel for Trainium2 (8 NeuronCores).

Input:  fp64_pulse (1048576, 64) float32 of {0,1} bits (fp64, MSB first).
Output: (1048576, 32) float32 of {0,1} bits (fp32 conversion result).

Strategy (pure data parallel over batch, 131072 rows/core):
  - host packs the 64 {0,1}-floats of each row into the two 32-bit words
    of the IEEE-754 double they spell (hi = sign/exp/mant[0:20],
    lo = mant[20:52]) -- 8 bytes/row instead of 256 (32x less traffic),
  - DVE stock ops do the shift/mask field extraction,
  - four custom fused DVE ops (multi-ALU-stage uop programs) compute the
    round-to-nearest-even increment, the final exponent byte with
    overflow/underflow clamping, the masked mantissa field, and the NaN
    quiet-bit term -- each replacing 4-7 single-ALU instructions,
  - the Pool engine (exact int32 arithmetic) assembles the final fp32
    word from the disjoint bit-field values,
  - the Activation engine produces the sign-bit term via a scaled copy,
  - host unpacks the fp32 words back to the (B, 32) {0,1} float layout.

Engine semantics (hardware-probed):
  - DVE stock: shifts/bitwise exact on i32; arith/compares in fp32
    (exact below 2^24).  - Pool tensor_tensor: exact int32 (wraps).
  - Custom DVE uops: i32 ports value-convert to fp32 and back;
    arith/compare/min/max/select stages only (no shifts).
"""
import numpy as np

from concourse import bacc, mybir
from concourse import dve_ops
from concourse.dve_ops import DveOp, OPS, _SUB_OPCODE_FOR_NAME
from concourse.dve_spec import (Spec, Src0, Src1, C0, C1, C2, Zero, One,
                                lower, AluOp, Bin, minn, maxx, ne)
from concourse.dve_spec import _has_src1 as has_src1
from concourse.dve_uop import DveOpSpec
from concourse.tile import TileContext
from concourse.bass_utils import run_bass_kernel_spmd

AOT = mybir.AluOpType
AFT = mybir.ActivationFunctionType
I32 = mybir.dt.int32

B = 1_048_576
N_CORES = 8
B_CORE = B // N_CORES          # 131072
P = 128                        # partitions
NI = B_CORE // P               # 1024 inner rows per partition

_CACHE = {}


def _register(name, body):
    if name in _SUB_OPCODE_FOR_NAME:
        return next(o for o in OPS if o.name == name)
    spec = Spec(body=body)
    row = 1 + len(OPS)
    _SUB_OPCODE_FOR_NAME[name] = row
    ds = DveOpSpec(name=name, opcode=row, uops=lower(spec, ver="v3"),
                   rd1_en=has_src1(spec))
    op = DveOp(name, spec, False, {"v3": ds.sha("v3")})
    OPS.append(op)
    dve_ops.CUSTOM_DVE_SPECS[name] = spec
    return op


IS_GE, IS_GT, IS_EQ = AluOp.IS_GE, AluOp.IS_GT, AluOp.IS_EQ

# FP64_M: in0 = mS = S01*2^23 + m23, in1 = RL = L*2 + R.
#   M = m23 + (R & (S | L)) = m23 + (RL==3) + (RL==1)*S   [C0 = 2^23, C1 = 3]
_S = Bin(IS_GE, Src0, C0)
_m23v = Src0 - _S * C0
_ru = Bin(IS_EQ, Src1, C1) + Bin(IS_EQ, Src1, One) * _S
FP64_M_BODY = _m23v + _ru

# FP64_E8S: in0 = e, in1 = M.  [C0 = 896, C1 = 2^23, C2 = 255]
#   E8 = min((e >= 897) * (e - 896 + cry), 255); out = E8 << 23
_e2m = Src0 - C0
_cry = Bin(IS_GE, Src1, C1)
_ge = Bin(IS_GE, _e2m, One)
FP64_E8S_BODY = minn(_ge * (_e2m + _cry), C2) * C1

# FP64_MN: in0 = M, in1 = e.  [C0 = 2^23, C1 = 897, C2 = 1150]
#   mant = M - cry*2^23; nrm = (e>=897) - (e>1150); out = nrm * mant
_mant = Src0 - Bin(IS_GE, Src0, C0) * C0
_nrm = Bin(IS_GE, Src1, C1) - Bin(IS_GT, Src1, C2)
FP64_MN_BODY = _nrm * _mant

# FP64_NAN: in0 = e, in1 = anyv (any nonzero int32 <=> mantissa nonzero).
#   out = (e == 2047) * (anyv != 0) * 2^22   [C0 = 2047, C1 = 2^22]
FP64_NAN_BODY = Bin(IS_EQ, Src0, C0) * ne(Src1, Zero) * C1

OP_M = _register("FP64_M", FP64_M_BODY)
OP_E8S = _register("FP64_E8S", FP64_E8S_BODY)
OP_MN = _register("FP64_MN", FP64_MN_BODY)
OP_NAN = _register("FP64_NAN", FP64_NAN_BODY)


def _build(n_chunks=2):
    assert NI % n_chunks == 0
    CH = NI // n_chunks
    nc = bacc.Bacc("TRN2")
    hi_d = nc.dram_tensor("hi", [B_CORE, 1], I32, kind="ExternalInput")
    lo_d = nc.dram_tensor("lo", [B_CORE, 1], I32, kind="ExternalInput")
    y_d = nc.dram_tensor("y", [B_CORE, 1], I32, kind="ExternalOutput")

    hi_r = hi_d.ap().rearrange("(p n) d -> p (n d)", p=P)   # [128, 1024]
    lo_r = lo_d.ap().rearrange("(p n) d -> p (n d)", p=P)
    y_r = y_d.ap().rearrange("(p n) d -> p (n d)", p=P)

    with TileContext(nc) as tc:
        with (
            tc.tile_pool(name="io", bufs=2) as io,
            tc.tile_pool(name="sc", bufs=2) as sc,
        ):
            for ci in range(n_chunks):
                off = ci * CH

                def t(name):
                    return sc.tile([P, CH], I32, tag=name, name=name)[:, :]

                hin = io.tile([P, CH], I32, tag="hi", name="hin")
                lin = io.tile([P, CH], I32, tag="lo", name="lin")
                nc.sync.dma_start(hin[:, :], hi_r[:, off:off + CH])
                nc.sync.dma_start(lin[:, :], lo_r[:, off:off + CH])
                hi = hin[:, :]
                lo = lin[:, :]
                V, G, A = nc.vector, nc.gpsimd, nc.scalar

                # --- DVE stock: field extraction (shift/mask, exact) ---
                e = t("e")
                V.tensor_scalar(e, hi, 20, 0x7FF, AOT.logical_shift_right,
                                AOT.bitwise_and)
                mh8 = t("mh8")
                V.tensor_scalar(mh8, hi, 0xFFFFF, 3, AOT.bitwise_and,
                                AOT.logical_shift_left)
                lo29 = t("lo29")
                V.tensor_scalar(lo29, lo, 29, None, AOT.logical_shift_right)
                s4 = t("s4")
                V.tensor_scalar(s4, lo, 4, None, AOT.logical_shift_left)
                RL = t("RL")
                V.tensor_scalar(RL, lo, 28, 3, AOT.logical_shift_right,
                                AOT.bitwise_and)
                anyv = t("anyv")
                V.tensor_tensor(anyv, mh8, lo, AOT.bitwise_or)

                # --- Pool: exact int32 arithmetic ---
                m23 = t("m23")
                G.tensor_tensor(m23, mh8, lo29, AOT.add)   # disjoint bits
                S01 = t("S01")
                V.tensor_scalar(S01, s4, 0, None, AOT.not_equal)
                s01 = t("s01")
                G.tensor_scalar(s01, hi, 0, None, AOT.is_lt)

                # mS = S01*2^23 + m23 (< 2^24, fp32-exact on DVE)
                mS = t("mS")
                V.scalar_tensor_tensor(mS, S01, 8388608, m23, AOT.mult,
                                       AOT.add)

                # --- fused DVE ops ---
                M = t("M")
                V._custom_dve(OP_M, out=M, in0=mS, in1=RL,
                              s0=8388608.0, s1=3.0)
                E8s = t("E8s")
                V._custom_dve(OP_E8S, out=E8s, in0=e, in1=M,
                              s0=896.0, s1=8388608.0, imm2=255.0)
                Mn = t("Mn")
                V._custom_dve(OP_MN, out=Mn, in0=M, in1=e,
                              s0=8388608.0, s1=897.0, imm2=1150.0)
                nanv = t("nanv")
                V._custom_dve(OP_NAN, out=nanv, in0=e, in1=anyv,
                              s0=2047.0, s1=4194304.0)

                # --- Act: sign term = s01 * -2^31 (saturating convert) ---
                sgnv = t("sgnv")
                A.activation(sgnv, s01, AFT.Copy, scale=-2147483648.0)

                # --- Pool: final assembly (exact int32 adds) ---
                o1 = t("o1")
                G.tensor_tensor(o1, E8s, Mn, AOT.add)
                o2 = t("o2")
                G.tensor_tensor(o2, o1, nanv, AOT.add)
                out = t("out")
                G.tensor_tensor(out, o2, sgnv, AOT.add)

                nc.sync.dma_start(y_r[:, off:off + CH], out)

    nc.compile()
    return nc


def _get_nc():
    if "nc" not in _CACHE:
        _CACHE["nc"] = _build()
    return _CACHE["nc"]


def _pack_inputs(x):
    """(B, 64) {0,1} float32 -> hi, lo int32 arrays of shape (B, 1)."""
    bits = x != 0
    pk = np.packbits(bits, axis=1)                  # (B, 8) MSB-first
    w = pk.view(">u4").astype(np.uint32)            # (B, 2) native
    hi = np.ascontiguousarray(w[:, 0]).view(np.int32).reshape(-1, 1)
    lo = np.ascontiguousarray(w[:, 1]).view(np.int32).reshape(-1, 1)
    return hi, lo


def _unpack_output(words):
    """(B, 1) int32 fp32 words -> (B, 32) float32 of {0,1} bits."""
    ob = words.reshape(-1).view(np.uint32).byteswap().view(np.uint8)
    return np.unpackbits(ob.reshape(-1, 4), axis=1).astype(np.float32)


def make_in_maps(x):
    hi, lo = _pack_inputs(np.ascontiguousarray(x, dtype=np.float32))
    return [
        {"hi": hi[c * B_CORE:(c + 1) * B_CORE],
         "lo": lo[c * B_CORE:(c + 1) * B_CORE]}
        for c in range(N_CORES)
    ]


def kernel(fp64_pulse: np.ndarray) -> np.ndarray:
    assert fp64_pulse.shape == (B, 64)
    nc = _get_nc()
    in_maps = make_in_maps(fp64_pulse)
    res = run_bass_kernel_spmd(nc, in_maps, core_ids=list(range(N_CORES)))
    words = np.concatenate([r["y"] for r in res.results], axis=0)
    return _unpack_output(words)


# revision 11
# speedup vs baseline: 4.9639x; 1.2941x over previous
"""FP64->FP32 bit-circuit converter kernel for Trainium2 (8 NeuronCores).

Input:  fp64_pulse (1048576, 64) float32 of {0,1} bits (fp64, MSB first).
Output: (1048576, 32) float32 of {0,1} bits (fp32 conversion result).

Strategy (pure data parallel over batch, 131072 rows/core):
  - host packs the 64 {0,1}-floats of each row into the two 32-bit words
    of the IEEE-754 double they spell (hi = sign/exp/mant[0:20],
    lo = mant[20:52]) -- 8 bytes/row instead of 256 (32x less traffic),
  - DVE stock ops do the shift/mask field extraction,
  - four custom fused DVE ops (multi-ALU-stage uop programs) compute the
    round-to-nearest-even increment, the final exponent byte with
    overflow/underflow clamping, the masked mantissa field, and the NaN
    quiet-bit term -- each replacing 4-7 single-ALU instructions,
  - the Pool engine (exact int32 arithmetic) assembles the final fp32
    word from the disjoint bit-field values,
  - the Activation engine produces the sign-bit term via a scaled copy,
  - host unpacks the fp32 words back to the (B, 32) {0,1} float layout.

Engine semantics (hardware-probed):
  - DVE stock: shifts/bitwise exact on i32; arith/compares in fp32
    (exact below 2^24).  - Pool tensor_tensor: exact int32 (wraps).
  - Custom DVE uops: i32 ports value-convert to fp32 and back;
    arith/compare/min/max/select stages only (no shifts).
"""
import numpy as np

from concourse import bacc, mybir
from concourse import dve_ops
from concourse.dve_ops import DveOp, OPS, _SUB_OPCODE_FOR_NAME
from concourse.dve_spec import (Spec, Src0, Src1, C0, C1, C2, Zero, One,
                                lower, AluOp, Bin, minn, maxx, ne)
from concourse.dve_spec import _has_src1 as has_src1
from concourse.dve_uop import DveOpSpec
from concourse.tile import TileContext
from concourse.bass_utils import run_bass_kernel_spmd

AOT = mybir.AluOpType
AFT = mybir.ActivationFunctionType
I32 = mybir.dt.int32

B = 1_048_576
N_CORES = 8
B_CORE = B // N_CORES          # 131072
P = 128                        # partitions
NI = B_CORE // P               # 1024 inner rows per partition

_CACHE = {}


def _register(name, body):
    if name in _SUB_OPCODE_FOR_NAME:
        return next(o for o in OPS if o.name == name)
    spec = Spec(body=body)
    row = 1 + len(OPS)
    _SUB_OPCODE_FOR_NAME[name] = row
    ds = DveOpSpec(name=name, opcode=row, uops=lower(spec, ver="v3"),
                   rd1_en=has_src1(spec))
    op = DveOp(name, spec, False, {"v3": ds.sha("v3")})
    OPS.append(op)
    dve_ops.CUSTOM_DVE_SPECS[name] = spec
    return op


IS_GE, IS_GT, IS_EQ = AluOp.IS_GE, AluOp.IS_GT, AluOp.IS_EQ

# FP64_M: in0 = mS = S01*2^23 + m23, in1 = RL = L*2 + R.
#   M = m23 + (R & (S | L)) = m23 + (RL==3) + (RL==1)*S   [C0 = 2^23, C1 = 3]
_S = Bin(IS_GE, Src0, C0)
_m23v = Src0 - _S * C0
_ru = Bin(IS_EQ, Src1, C1) + Bin(IS_EQ, Src1, One) * _S
FP64_M_BODY = _m23v + _ru

# FP64_E8S: in0 = e, in1 = M.  [C0 = 896, C1 = 2^23, C2 = 255]
#   E8 = min((e >= 897) * (e - 896 + cry), 255); out = E8 << 23
_e2m = Src0 - C0
_cry = Bin(IS_GE, Src1, C1)
_ge = Bin(IS_GE, _e2m, One)
FP64_E8S_BODY = minn(_ge * (_e2m + _cry), C2) * C1

# FP64_MN: in0 = M, in1 = e.  [C0 = 2^23, C1 = 897, C2 = 1150]
#   mant = M - cry*2^23; nrm = (e>=897) - (e>1150); out = nrm * mant
_mant = Src0 - Bin(IS_GE, Src0, C0) * C0
_nrm = Bin(IS_GE, Src1, C1) - Bin(IS_GT, Src1, C2)
FP64_MN_BODY = _nrm * _mant

# FP64_NAN: in0 = e, in1 = anyv (any nonzero int32 <=> mantissa nonzero).
#   out = (e == 2047) * (anyv != 0) * 2^22   [C0 = 2047, C1 = 2^22]
FP64_NAN_BODY = Bin(IS_EQ, Src0, C0) * ne(Src1, Zero) * C1

OP_M = _register("FP64_M", FP64_M_BODY)
OP_E8S = _register("FP64_E8S", FP64_E8S_BODY)
OP_MN = _register("FP64_MN", FP64_MN_BODY)
OP_NAN = _register("FP64_NAN", FP64_NAN_BODY)


def _build(n_chunks=4):
    assert NI % n_chunks == 0
    CH = NI // n_chunks
    nc = bacc.Bacc("TRN2")
    hi_d = nc.dram_tensor("hi", [B_CORE, 1], I32, kind="ExternalInput")
    lo_d = nc.dram_tensor("lo", [B_CORE, 1], I32, kind="ExternalInput")
    y_d = nc.dram_tensor("y", [B_CORE, 1], I32, kind="ExternalOutput")

    hi_r = hi_d.ap().rearrange("(p n) d -> p (n d)", p=P)   # [128, 1024]
    lo_r = lo_d.ap().rearrange("(p n) d -> p (n d)", p=P)
    y_r = y_d.ap().rearrange("(p n) d -> p (n d)", p=P)

    with TileContext(nc) as tc:
        with (
            tc.tile_pool(name="io", bufs=2) as io,
            tc.tile_pool(name="sc", bufs=2) as sc,
        ):
            for ci in range(n_chunks):
                off = ci * CH

                def t(name):
                    return sc.tile([P, CH], I32, tag=name, name=name)[:, :]

                hin = io.tile([P, CH], I32, tag="hi", name="hin")
                lin = io.tile([P, CH], I32, tag="lo", name="lin")
                nc.sync.dma_start(hin[:, :], hi_r[:, off:off + CH])
                nc.sync.dma_start(lin[:, :], lo_r[:, off:off + CH])
                hi = hin[:, :]
                lo = lin[:, :]
                V, G, A = nc.vector, nc.gpsimd, nc.scalar

                # --- DVE stock: field extraction (shift/mask, exact) ---
                e = t("e")
                V.tensor_scalar(e, hi, 20, 0x7FF, AOT.logical_shift_right,
                                AOT.bitwise_and)
                mh8 = t("mh8")
                V.tensor_scalar(mh8, hi, 0xFFFFF, 3, AOT.bitwise_and,
                                AOT.logical_shift_left)
                lo29 = t("lo29")
                V.tensor_scalar(lo29, lo, 29, None, AOT.logical_shift_right)
                s4 = t("s4")
                V.tensor_scalar(s4, lo, 4, None, AOT.logical_shift_left)
                RL = t("RL")
                V.tensor_scalar(RL, lo, 28, 3, AOT.logical_shift_right,
                                AOT.bitwise_and)
                s01 = t("s01")
                V.tensor_scalar(s01, hi, 31, None, AOT.logical_shift_right)

                # --- Pool: exact int32 arithmetic ---
                m23 = t("m23")
                G.tensor_tensor(m23, mh8, lo29, AOT.add)   # disjoint bits
                S01 = t("S01")
                V.tensor_scalar(S01, s4, 0, None, AOT.not_equal)

                # mS = S01*2^23 + m23 (< 2^24, fp32-exact on DVE)
                mS = t("mS")
                V.scalar_tensor_tensor(mS, S01, 8388608, m23, AOT.mult,
                                       AOT.add)
                # anyv-equivalent: mS + RL != 0  <=>  mantissa != 0
                # (L is subsumed by m23; fp32 rounding preserves nonzero-ness)
                anyv = t("anyv")
                V.scalar_tensor_tensor(anyv, mS, 1, RL, AOT.mult, AOT.add)

                # --- fused DVE ops ---
                M = t("M")
                V._custom_dve(OP_M, out=M, in0=mS, in1=RL,
                              s0=8388608.0, s1=3.0)
                E8s = t("E8s")
                V._custom_dve(OP_E8S, out=E8s, in0=e, in1=M,
                              s0=896.0, s1=8388608.0, imm2=255.0)
                Mn = t("Mn")
                V._custom_dve(OP_MN, out=Mn, in0=M, in1=e,
                              s0=8388608.0, s1=897.0, imm2=1150.0)
                nanv = t("nanv")
                V._custom_dve(OP_NAN, out=nanv, in0=e, in1=anyv,
                              s0=2047.0, s1=4194304.0)

                # --- Act: sign term = s01 * -2^31 (saturating convert) ---
                sgnv = t("sgnv")
                A.activation(sgnv, s01, AFT.Copy, scale=-2147483648.0)

                # --- Pool: final assembly (exact int32 adds) ---
                o1 = t("o1")
                G.tensor_tensor(o1, E8s, Mn, AOT.add)
                o2 = t("o2")
                G.tensor_tensor(o2, o1, nanv, AOT.add)
                out = t("out")
                G.tensor_tensor(out, o2, sgnv, AOT.add)

                nc.sync.dma_start(y_r[:, off:off + CH], out)

    nc.compile()
    return nc


def _get_nc():
    if "nc" not in _CACHE:
        _CACHE["nc"] = _build()
    return _CACHE["nc"]


def _pack_inputs(x):
    """(B, 64) {0,1} float32 -> hi, lo int32 arrays of shape (B, 1)."""
    bits = x != 0
    pk = np.packbits(bits, axis=1)                  # (B, 8) MSB-first
    w = pk.view(">u4").astype(np.uint32)            # (B, 2) native
    hi = np.ascontiguousarray(w[:, 0]).view(np.int32).reshape(-1, 1)
    lo = np.ascontiguousarray(w[:, 1]).view(np.int32).reshape(-1, 1)
    return hi, lo


def _unpack_output(words):
    """(B, 1) int32 fp32 words -> (B, 32) float32 of {0,1} bits."""
    ob = words.reshape(-1).view(np.uint32).byteswap().view(np.uint8)
    return np.unpackbits(ob.reshape(-1, 4), axis=1).astype(np.float32)


def make_in_maps(x):
    hi, lo = _pack_inputs(np.ascontiguousarray(x, dtype=np.float32))
    return [
        {"hi": hi[c * B_CORE:(c + 1) * B_CORE],
         "lo": lo[c * B_CORE:(c + 1) * B_CORE]}
        for c in range(N_CORES)
    ]


def kernel(fp64_pulse: np.ndarray) -> np.ndarray:
    assert fp64_pulse.shape == (B, 64)
    nc = _get_nc()
    in_maps = make_in_maps(fp64_pulse)
    res = run_bass_kernel_spmd(nc, in_maps, core_ids=list(range(N_CORES)))
    words = np.concatenate([r["y"] for r in res.results], axis=0)
    return _unpack_output(words)


# revision 14
# speedup vs baseline: 5.0890x; 1.0252x over previous
"""FP64->FP32 bit-circuit converter kernel for Trainium2 (8 NeuronCores).

Input:  fp64_pulse (1048576, 64) float32 of {0,1} bits (fp64, MSB first).
Output: (1048576, 32) float32 of {0,1} bits (fp32 conversion result).

Strategy (pure data parallel over batch, 131072 rows/core):
  - host packs the 64 {0,1}-floats of each row into the two 32-bit words
    of the IEEE-754 double they spell (hi = sign/exp/mant[0:20],
    lo = mant[20:52]) -- 8 bytes/row instead of 256 (32x less traffic),
  - DVE stock ops do the shift/mask field extraction,
  - four custom fused DVE ops (multi-ALU-stage uop programs) compute the
    round-to-nearest-even increment, the final exponent byte with
    overflow/underflow clamping, the masked mantissa field, and the NaN
    quiet-bit term -- each replacing 4-7 single-ALU instructions,
  - the Pool engine (exact int32 arithmetic) assembles the final fp32
    word from the disjoint bit-field values,
  - the Activation engine produces the sign-bit term via a scaled copy,
  - host unpacks the fp32 words back to the (B, 32) {0,1} float layout.

Engine semantics (hardware-probed):
  - DVE stock: shifts/bitwise exact on i32; arith/compares in fp32
    (exact below 2^24).  - Pool tensor_tensor: exact int32 (wraps).
  - Custom DVE uops: i32 ports value-convert to fp32 and back;
    arith/compare/min/max/select stages only (no shifts).
"""
import numpy as np

from concourse import bacc, mybir
from concourse import dve_ops
from concourse.dve_ops import DveOp, OPS, _SUB_OPCODE_FOR_NAME
from concourse.dve_spec import (Spec, Src0, Src1, C0, C1, C2, Zero, One,
                                lower, AluOp, Bin, minn, maxx, ne)
from concourse.dve_spec import _has_src1 as has_src1
from concourse.dve_uop import DveOpSpec
from concourse.tile import TileContext
from concourse.bass_utils import run_bass_kernel_spmd

AOT = mybir.AluOpType
AFT = mybir.ActivationFunctionType
I32 = mybir.dt.int32

B = 1_048_576
N_CORES = 8
B_CORE = B // N_CORES          # 131072
P = 128                        # partitions
NI = B_CORE // P               # 1024 inner rows per partition

_CACHE = {}


def _register(name, body):
    if name in _SUB_OPCODE_FOR_NAME:
        return next(o for o in OPS if o.name == name)
    spec = Spec(body=body)
    row = 1 + len(OPS)
    _SUB_OPCODE_FOR_NAME[name] = row
    ds = DveOpSpec(name=name, opcode=row, uops=lower(spec, ver="v3"),
                   rd1_en=has_src1(spec))
    op = DveOp(name, spec, False, {"v3": ds.sha("v3")})
    OPS.append(op)
    dve_ops.CUSTOM_DVE_SPECS[name] = spec
    return op


IS_GE, IS_GT, IS_EQ = AluOp.IS_GE, AluOp.IS_GT, AluOp.IS_EQ

# FP64_M: in0 = mS = S01*2^23 + m23, in1 = RL = L*2 + R.
#   M = m23 + (R & (S | L)) = m23 + (RL==3) + (RL==1)*S   [C0 = 2^23, C1 = 3]
_S = Bin(IS_GE, Src0, C0)
_m23v = Src0 - _S * C0
_ru = Bin(IS_EQ, Src1, C1) + Bin(IS_EQ, Src1, One) * _S
FP64_M_BODY = _m23v + _ru

# FP64_E8S: in0 = e, in1 = M.  [C0 = 896, C1 = 2^23, C2 = 255]
#   E8 = min((e >= 897) * (e - 896 + cry), 255); out = E8 << 23
_e2m = Src0 - C0
_cry = Bin(IS_GE, Src1, C1)
_ge = Bin(IS_GE, _e2m, One)
FP64_E8S_BODY = minn(_ge * (_e2m + _cry), C2) * C1

# FP64_MN: in0 = M, in1 = e.  [C0 = 2^23, C1 = 897, C2 = 1150]
#   mant = M - cry*2^23; nrm = (e>=897) - (e>1150); out = nrm * mant
_mant = Src0 - Bin(IS_GE, Src0, C0) * C0
_nrm = Bin(IS_GE, Src1, C1) - Bin(IS_GT, Src1, C2)
FP64_MN_BODY = _nrm * _mant

# FP64_NAN: in0 = e, in1 = anyv (any nonzero int32 <=> mantissa nonzero).
#   out = (e == 2047) * (anyv != 0) * 2^22   [C0 = 2047, C1 = 2^22]
FP64_NAN_BODY = Bin(IS_EQ, Src0, C0) * ne(Src1, Zero) * C1

# FP64_MAC: out = Src0 * C0 + Src1 (values < 2^24, fp32-exact)
FP64_MAC_BODY = Src0 * C0 + Src1

# FP64_MS: out = (Src0 != 0) * C0 + Src1  (sticky flag fold: in0 = s4 raw,
#   nonzero-ness survives the fp32 value cast; C0 = 2^23, in1 = m23)
FP64_MS_BODY = ne(Src0, Zero) * C0 + Src1

OP_M = _register("FP64_M", FP64_M_BODY)
OP_E8S = _register("FP64_E8S", FP64_E8S_BODY)
OP_MN = _register("FP64_MN", FP64_MN_BODY)
OP_NAN = _register("FP64_NAN", FP64_NAN_BODY)
OP_MAC = _register("FP64_MAC", FP64_MAC_BODY)
OP_MS = _register("FP64_MS", FP64_MS_BODY)


def _build(n_chunks=4):
    assert NI % n_chunks == 0
    CH = NI // n_chunks
    nc = bacc.Bacc("TRN2")
    hi_d = nc.dram_tensor("hi", [B_CORE, 1], I32, kind="ExternalInput")
    lo_d = nc.dram_tensor("lo", [B_CORE, 1], I32, kind="ExternalInput")
    y_d = nc.dram_tensor("y", [B_CORE, 1], I32, kind="ExternalOutput")

    hi_r = hi_d.ap().rearrange("(p n) d -> p (n d)", p=P)   # [128, 1024]
    lo_r = lo_d.ap().rearrange("(p n) d -> p (n d)", p=P)
    y_r = y_d.ap().rearrange("(p n) d -> p (n d)", p=P)

    with TileContext(nc) as tc:
        with (
            tc.tile_pool(name="io", bufs=2) as io,
            tc.tile_pool(name="sc", bufs=2) as sc,
        ):
            for ci in range(n_chunks):
                off = ci * CH

                def t(name):
                    return sc.tile([P, CH], I32, tag=name, name=name)[:, :]

                hin = io.tile([P, CH], I32, tag="hi", name="hin")
                lin = io.tile([P, CH], I32, tag="lo", name="lin")
                nc.sync.dma_start(hin[:, :], hi_r[:, off:off + CH])
                nc.sync.dma_start(lin[:, :], lo_r[:, off:off + CH])
                hi = hin[:, :]
                lo = lin[:, :]
                V, G, A = nc.vector, nc.gpsimd, nc.scalar

                # --- DVE stock: field extraction (shift/mask, exact) ---
                e = t("e")
                V.tensor_scalar(e, hi, 20, 0x7FF, AOT.logical_shift_right,
                                AOT.bitwise_and)
                mh8 = t("mh8")
                V.tensor_scalar(mh8, hi, 0xFFFFF, 3, AOT.bitwise_and,
                                AOT.logical_shift_left)
                lo29 = t("lo29")
                V.tensor_scalar(lo29, lo, 29, None, AOT.logical_shift_right)
                s4 = t("s4")
                V.tensor_scalar(s4, lo, 4, None, AOT.logical_shift_left)
                RL = t("RL")
                V.tensor_scalar(RL, lo, 28, 3, AOT.logical_shift_right,
                                AOT.bitwise_and)
                s01 = t("s01")
                V.tensor_scalar(s01, hi, 31, None, AOT.logical_shift_right)

                # m23 = mh8 + lo29 (disjoint bits, < 2^24: fp32-exact MAC)
                m23 = t("m23")
                V._custom_dve(OP_MAC, out=m23, in0=mh8, in1=lo29, s0=1.0)
                # mS = (s4 != 0)*2^23 + m23 (sticky flag folded in)
                mS = t("mS")
                V._custom_dve(OP_MS, out=mS, in0=s4, in1=m23, s0=8388608.0)
                # anyv-equivalent: mS + RL != 0  <=>  mantissa != 0
                # (L is subsumed by m23; fp32 rounding preserves nonzero-ness)
                anyv = t("anyv")
                V._custom_dve(OP_MAC, out=anyv, in0=mS, in1=RL, s0=1.0)

                # --- fused DVE ops ---
                M = t("M")
                V._custom_dve(OP_M, out=M, in0=mS, in1=RL,
                              s0=8388608.0, s1=3.0)
                E8s = t("E8s")
                V._custom_dve(OP_E8S, out=E8s, in0=e, in1=M,
                              s0=896.0, s1=8388608.0, imm2=255.0)
                Mn = t("Mn")
                V._custom_dve(OP_MN, out=Mn, in0=M, in1=e,
                              s0=8388608.0, s1=897.0, imm2=1150.0)
                nanv = t("nanv")
                V._custom_dve(OP_NAN, out=nanv, in0=e, in1=anyv,
                              s0=2047.0, s1=4194304.0)

                # --- Act: sign term = s01 * -2^31 (saturating convert) ---
                sgnv = t("sgnv")
                A.activation(sgnv, s01, AFT.Copy, scale=-2147483648.0)

                # --- Pool: final assembly (exact int32 adds) ---
                ns = t("ns")
                G.tensor_tensor(ns, nanv, sgnv, AOT.add)
                o1 = t("o1")
                G.tensor_tensor(o1, E8s, Mn, AOT.add)
                out = t("out")
                G.tensor_tensor(out, o1, ns, AOT.add)

                nc.sync.dma_start(y_r[:, off:off + CH], out)

    nc.compile()
    return nc


def _get_nc():
    if "nc" not in _CACHE:
        _CACHE["nc"] = _build()
    return _CACHE["nc"]


def _pack_inputs(x):
    """(B, 64) {0,1} float32 -> hi, lo int32 arrays of shape (B, 1)."""
    bits = x != 0
    pk = np.packbits(bits, axis=1)                  # (B, 8) MSB-first
    w = pk.view(">u4").astype(np.uint32)            # (B, 2) native
    hi = np.ascontiguousarray(w[:, 0]).view(np.int32).reshape(-1, 1)
    lo = np.ascontiguousarray(w[:, 1]).view(np.int32).reshape(-1, 1)
    return hi, lo


def _unpack_output(words):
    """(B, 1) int32 fp32 words -> (B, 32) float32 of {0,1} bits."""
    ob = words.reshape(-1).view(np.uint32).byteswap().view(np.uint8)
    return np.unpackbits(ob.reshape(-1, 4), axis=1).astype(np.float32)


def make_in_maps(x):
    hi, lo = _pack_inputs(np.ascontiguousarray(x, dtype=np.float32))
    return [
        {"hi": hi[c * B_CORE:(c + 1) * B_CORE],
         "lo": lo[c * B_CORE:(c + 1) * B_CORE]}
        for c in range(N_CORES)
    ]


def kernel(fp64_pulse: np.ndarray) -> np.ndarray:
    assert fp64_pulse.shape == (B, 64)
    nc = _get_nc()
    in_maps = make_in_maps(fp64_pulse)
    res = run_bass_kernel_spmd(nc, in_maps, core_ids=list(range(N_CORES)))
    words = np.concatenate([r["y"] for r in res.results], axis=0)
    return _unpack_output(words)


# revision 15
# speedup vs baseline: 5.6643x; 1.1130x over previous
"""FP64->FP32 bit-circuit converter kernel for Trainium2 (8 NeuronCores).

Input:  fp64_pulse (1048576, 64) float32 of {0,1} bits (fp64, MSB first).
Output: (1048576, 32) float32 of {0,1} bits (fp32 conversion result).

Strategy (pure data parallel over batch, 131072 rows/core):
  - host packs the 64 {0,1}-floats of each row into the two 32-bit words
    of the IEEE-754 double they spell (hi = sign/exp/mant[0:20],
    lo = mant[20:52]) -- 8 bytes/row instead of 256 (32x less traffic),
  - DVE stock ops do the shift/mask field extraction,
  - four custom fused DVE ops (multi-ALU-stage uop programs) compute the
    round-to-nearest-even increment, the final exponent byte with
    overflow/underflow clamping, the masked mantissa field, and the NaN
    quiet-bit term -- each replacing 4-7 single-ALU instructions,
  - the Pool engine (exact int32 arithmetic) assembles the final fp32
    word from the disjoint bit-field values,
  - the Activation engine produces the sign-bit term via a scaled copy,
  - host unpacks the fp32 words back to the (B, 32) {0,1} float layout.

Engine semantics (hardware-probed):
  - DVE stock: shifts/bitwise exact on i32; arith/compares in fp32
    (exact below 2^24).  - Pool tensor_tensor: exact int32 (wraps).
  - Custom DVE uops: i32 ports value-convert to fp32 and back;
    arith/compare/min/max/select stages only (no shifts).
"""
import numpy as np

from concourse import bacc, mybir
from concourse import dve_ops
from concourse.dve_ops import DveOp, OPS, _SUB_OPCODE_FOR_NAME
from concourse.dve_spec import (Spec, Src0, Src1, C0, C1, C2, Zero, One,
                                lower, AluOp, Bin, minn, maxx, ne)
from concourse.dve_spec import _has_src1 as has_src1
from concourse.dve_uop import DveOpSpec
from concourse.tile import TileContext
from concourse.bass_utils import run_bass_kernel_spmd

AOT = mybir.AluOpType
AFT = mybir.ActivationFunctionType
I32 = mybir.dt.int32

B = 1_048_576
N_CORES = 8
B_CORE = B // N_CORES          # 131072
P = 128                        # partitions
NI = B_CORE // P               # 1024 inner rows per partition

_CACHE = {}


def _register(name, body):
    if name in _SUB_OPCODE_FOR_NAME:
        return next(o for o in OPS if o.name == name)
    spec = Spec(body=body)
    row = 1 + len(OPS)
    _SUB_OPCODE_FOR_NAME[name] = row
    ds = DveOpSpec(name=name, opcode=row, uops=lower(spec, ver="v3"),
                   rd1_en=has_src1(spec))
    op = DveOp(name, spec, False, {"v3": ds.sha("v3")})
    OPS.append(op)
    dve_ops.CUSTOM_DVE_SPECS[name] = spec
    return op


IS_GE, IS_GT, IS_EQ = AluOp.IS_GE, AluOp.IS_GT, AluOp.IS_EQ

# FP64_M: in0 = mS = S01*2^23 + m23, in1 = RL = L*2 + R.
#   M = m23 + (R & (S | L)) = m23 + (RL==3) + (RL==1)*S   [C0 = 2^23, C1 = 3]
_S = Bin(IS_GE, Src0, C0)
_m23v = Src0 - _S * C0
_ru = Bin(IS_EQ, Src1, C1) + Bin(IS_EQ, Src1, One) * _S
FP64_M_BODY = _m23v + _ru

# FP64_E8S: in0 = e, in1 = M.  [C0 = 896, C1 = 2^23, C2 = 255]
#   E8 = min((e >= 897) * (e - 896 + cry), 255); out = E8 << 23
_e2m = Src0 - C0
_cry = Bin(IS_GE, Src1, C1)
_ge = Bin(IS_GE, _e2m, One)
FP64_E8S_BODY = minn(_ge * (_e2m + _cry), C2) * C1

# FP64_MN: in0 = M, in1 = e.  [C0 = 2^23, C1 = 897, C2 = 1150]
#   mant = M - cry*2^23; nrm = (e>=897) - (e>1150); out = nrm * mant
_mant = Src0 - Bin(IS_GE, Src0, C0) * C0
_nrm = Bin(IS_GE, Src1, C1) - Bin(IS_GT, Src1, C2)
FP64_MN_BODY = _nrm * _mant

# FP64_NAN: in0 = e, in1 = anyv (any nonzero int32 <=> mantissa nonzero).
#   out = (e == 2047) * (anyv != 0) * 2^22   [C0 = 2047, C1 = 2^22]
FP64_NAN_BODY = Bin(IS_EQ, Src0, C0) * ne(Src1, Zero) * C1

# FP64_MAC: out = Src0 * C0 + Src1 (values < 2^24, fp32-exact)
FP64_MAC_BODY = Src0 * C0 + Src1

# FP64_MS: out = (Src0 != 0) * C0 + Src1  (sticky flag fold: in0 = s4 raw,
#   nonzero-ness survives the fp32 value cast; C0 = 2^23, in1 = m23)
FP64_MS_BODY = ne(Src0, Zero) * C0 + Src1

OP_M = _register("FP64_M", FP64_M_BODY)
OP_E8S = _register("FP64_E8S", FP64_E8S_BODY)
OP_MN = _register("FP64_MN", FP64_MN_BODY)
OP_NAN = _register("FP64_NAN", FP64_NAN_BODY)
OP_MAC = _register("FP64_MAC", FP64_MAC_BODY)
OP_MS = _register("FP64_MS", FP64_MS_BODY)


def _build(n_chunks=2):
    assert NI % n_chunks == 0
    CH = NI // n_chunks
    nc = bacc.Bacc("TRN2")
    hi_d = nc.dram_tensor("hi", [B_CORE, 1], I32, kind="ExternalInput")
    lo_d = nc.dram_tensor("lo", [B_CORE, 1], I32, kind="ExternalInput")
    y_d = nc.dram_tensor("y", [B_CORE, 1], I32, kind="ExternalOutput")

    hi_r = hi_d.ap().rearrange("(p n) d -> p (n d)", p=P)   # [128, 1024]
    lo_r = lo_d.ap().rearrange("(p n) d -> p (n d)", p=P)
    y_r = y_d.ap().rearrange("(p n) d -> p (n d)", p=P)

    with TileContext(nc) as tc:
        with (
            tc.tile_pool(name="io", bufs=2) as io,
            tc.tile_pool(name="sc", bufs=2) as sc,
        ):
            for ci in range(n_chunks):
                off = ci * CH

                def t(name):
                    return sc.tile([P, CH], I32, tag=name, name=name)[:, :]

                hin = io.tile([P, CH], I32, tag="hi", name="hin")
                lin = io.tile([P, CH], I32, tag="lo", name="lin")
                nc.sync.dma_start(hin[:, :], hi_r[:, off:off + CH])
                nc.sync.dma_start(lin[:, :], lo_r[:, off:off + CH])
                hi = hin[:, :]
                lo = lin[:, :]
                V, G, A = nc.vector, nc.gpsimd, nc.scalar

                # --- DVE stock: field extraction (shift/mask, exact) ---
                e = t("e")
                V.tensor_scalar(e, hi, 20, 0x7FF, AOT.logical_shift_right,
                                AOT.bitwise_and)
                mh8 = t("mh8")
                V.tensor_scalar(mh8, hi, 0xFFFFF, 3, AOT.bitwise_and,
                                AOT.logical_shift_left)
                lo29 = t("lo29")
                V.tensor_scalar(lo29, lo, 29, None, AOT.logical_shift_right)
                s4 = t("s4")
                V.tensor_scalar(s4, lo, 4, None, AOT.logical_shift_left)
                RL = t("RL")
                V.tensor_scalar(RL, lo, 28, 3, AOT.logical_shift_right,
                                AOT.bitwise_and)
                s01 = t("s01")
                V.tensor_scalar(s01, hi, 31, None, AOT.logical_shift_right)

                # m23 = mh8 + lo29 (disjoint bits, < 2^24: fp32-exact MAC)
                m23 = t("m23")
                V._custom_dve(OP_MAC, out=m23, in0=mh8, in1=lo29, s0=1.0)
                # mS = (s4 != 0)*2^23 + m23 (sticky flag folded in)
                mS = t("mS")
                V._custom_dve(OP_MS, out=mS, in0=s4, in1=m23, s0=8388608.0)
                # anyv-equivalent: mS + RL != 0  <=>  mantissa != 0
                # (L is subsumed by m23; fp32 rounding preserves nonzero-ness)
                anyv = t("anyv")
                V._custom_dve(OP_MAC, out=anyv, in0=mS, in1=RL, s0=1.0)

                # --- fused DVE ops ---
                M = t("M")
                V._custom_dve(OP_M, out=M, in0=mS, in1=RL,
                              s0=8388608.0, s1=3.0)
                E8s = t("E8s")
                V._custom_dve(OP_E8S, out=E8s, in0=e, in1=M,
                              s0=896.0, s1=8388608.0, imm2=255.0)
                Mn = t("Mn")
                V._custom_dve(OP_MN, out=Mn, in0=M, in1=e,
                              s0=8388608.0, s1=897.0, imm2=1150.0)
                nanv = t("nanv")
                V._custom_dve(OP_NAN, out=nanv, in0=e, in1=anyv,
                              s0=2047.0, s1=4194304.0)

                # --- Act: sign term = s01 * -2^31 (saturating convert) ---
                sgnv = t("sgnv")
                A.activation(sgnv, s01, AFT.Copy, scale=-2147483648.0)

                # --- Pool: final assembly (exact int32 adds) ---
                ns = t("ns")
                G.tensor_tensor(ns, nanv, sgnv, AOT.add)
                o1 = t("o1")
                G.tensor_tensor(o1, E8s, Mn, AOT.add)
                out = t("out")
                G.tensor_tensor(out, o1, ns, AOT.add)

                nc.sync.dma_start(y_r[:, off:off + CH], out)

    nc.compile()
    return nc


def _get_nc():
    if "nc" not in _CACHE:
        _CACHE["nc"] = _build()
    return _CACHE["nc"]


def _pack_inputs(x):
    """(B, 64) {0,1} float32 -> hi, lo int32 arrays of shape (B, 1)."""
    bits = x != 0
    pk = np.packbits(bits, axis=1)                  # (B, 8) MSB-first
    w = pk.view(">u4").astype(np.uint32)            # (B, 2) native
    hi = np.ascontiguousarray(w[:, 0]).view(np.int32).reshape(-1, 1)
    lo = np.ascontiguousarray(w[:, 1]).view(np.int32).reshape(-1, 1)
    return hi, lo


def _unpack_output(words):
    """(B, 1) int32 fp32 words -> (B, 32) float32 of {0,1} bits."""
    ob = words.reshape(-1).view(np.uint32).byteswap().view(np.uint8)
    return np.unpackbits(ob.reshape(-1, 4), axis=1).astype(np.float32)


def make_in_maps(x):
    hi, lo = _pack_inputs(np.ascontiguousarray(x, dtype=np.float32))
    return [
        {"hi": hi[c * B_CORE:(c + 1) * B_CORE],
         "lo": lo[c * B_CORE:(c + 1) * B_CORE]}
        for c in range(N_CORES)
    ]


def kernel(fp64_pulse: np.ndarray) -> np.ndarray:
    assert fp64_pulse.shape == (B, 64)
    nc = _get_nc()
    in_maps = make_in_maps(fp64_pulse)
    res = run_bass_kernel_spmd(nc, in_maps, core_ids=list(range(N_CORES)))
    words = np.concatenate([r["y"] for r in res.results], axis=0)
    return _unpack_output(words)
